# revision 3
# baseline (speedup 1.0000x reference)
"""GCN encoder (2-layer GCNConv + embedding lookup) on 8 trn2 NeuronCores.

Strategy (graph-parallel, per sharding hint):
  - Nodes are sharded across the 8 cores by id (12500 each), then renumbered
    into "slots": each core packs its nodes into G groups of <=128 nodes with
    balanced edge counts (bin-packing on host), giving a uniform SPMD program.
  - norm folding: out = dis .* segsum(u[src]) + b with u = dis .* (x @ W),
    dis = deg^-1/2 (self-loops included as ordinary edges).
  - u1 = dis * embW1[type] (embW1 = emb@W1 folded on host) computed per shard,
    AllGather -> full u1 table; per-group edge gathers (dma_gather, int16 idx
    with +-32768 midpoint base trick, 2 tables) feed one-hot matmul
    aggregation accumulated in PSUM; layer-2 the same with u2 = dis*(h1@W2).
"""
import os
import sys

sys.path.insert(0, "/opt/trn_rl_repo")
import numpy as np

N_NODES = 100000
NCORE = 8
NPC = N_NODES // NCORE          # 12500 nodes per core
D1, D2 = 128, 64
NTYPES = 1000
G = 104                         # groups per core
W = 128                         # slots (nodes) per group
SLOTS = G * W                   # 13312 slots per core
TOTAL_SLOTS = NCORE * SLOTS     # 106496
BASE_A, BASE_B = 32768, 73728   # gather base rows (midpoint trick)
SPLIT_NODE = 4 * NPC            # src node < 50000 -> table A (cores 0-3)
PAD_DST = 999.0                 # dst_local sentinel -> zero one-hot row


# ---------------------------------------------------------------- host prep
def _pack_core(nodes, degA, degB):
    """Greedy 2-d balanced packing of `nodes` into G groups of <=W nodes.
    Returns list of per-group node arrays."""
    a = degA[nodes].astype(np.float64)
    b = degB[nodes].astype(np.float64)
    order = np.argsort(-(a + b), kind="stable")
    tA = max(a.sum() / G, 1.0)
    tB = max(b.sum() / G, 1.0)
    sA = np.zeros(G)
    sB = np.zeros(G)
    cnt = np.zeros(G, np.int64)
    assign = np.empty(len(nodes), np.int64)
    for i in order:
        cost = np.maximum((sA + a[i]) / tA, (sB + b[i]) / tB)
        cost[cnt >= W] = np.inf
        g = int(np.argmin(cost))
        assign[i] = g
        sA[g] += a[i]
        sB[g] += b[i]
        cnt[g] += 1
    groups = [nodes[assign == g] for g in range(G)]
    return groups


def preprocess(x_node_types, edge_index, emb, W1, b1, W2, b2):
    types = np.asarray(x_node_types).astype(np.int64)
    src = np.asarray(edge_index[0]).astype(np.int64)
    dst = np.asarray(edge_index[1]).astype(np.int64)
    loop = np.arange(N_NODES, dtype=np.int64)
    src_all = np.concatenate([src, loop])
    dst_all = np.concatenate([dst, loop])

    deg = np.bincount(dst_all, minlength=N_NODES).astype(np.float32)
    dis = (1.0 / np.sqrt(deg)).astype(np.float32)   # deg >= 1 (self loops)

    a_mask = src_all < SPLIT_NODE
    degA = np.bincount(dst_all[a_mask], minlength=N_NODES)
    degB = np.bincount(dst_all[~a_mask], minlength=N_NODES)

    # pack nodes -> slots
    slot_of = np.full(N_NODES, -1, np.int64)
    slot2node = np.full((NCORE, SLOTS), -1, np.int64)
    for c in range(NCORE):
        nodes = np.arange(c * NPC, (c + 1) * NPC, dtype=np.int64)
        groups = _pack_core(nodes, degA, degB)
        for g, gn in enumerate(groups):
            s0 = g * W
            slot_of[gn] = c * SLOTS + s0 + np.arange(len(gn))
            slot2node[c, s0:s0 + len(gn)] = gn

    # per-edge data
    e_srcslot = slot_of[src_all]
    e_dstslot = slot_of[dst_all]
    e_core = e_dstslot // SLOTS
    e_grp = (e_dstslot % SLOTS) // W
    e_dloc = e_dstslot % W
    e_sec = (src_all >= SPLIT_NODE).astype(np.int64)   # 0 = A, 1 = B

    bucket = (e_core * G + e_grp) * 2 + e_sec
    nb = NCORE * G * 2
    counts = np.bincount(bucket, minlength=nb)
    cA = counts.reshape(NCORE, G, 2)[:, :, 0]
    cB = counts.reshape(NCORE, G, 2)[:, :, 1]
    CA = int(np.ceil(cA.max() / 128))
    CB = int(np.ceil(cB.max() / 128))
    C = CA + CB

    # stable order by bucket; rank within bucket
    order = np.argsort(bucket, kind="stable")
    offs = np.zeros(nb + 1, np.int64)
    np.cumsum(counts, out=offs[1:])
    rank = np.arange(len(bucket)) - offs[bucket[order]]

    # padded per-(core,group,sec) slot position -> chunk & partition
    nsec = np.array([CA * 128, CB * 128])
    secbase = np.array([0, CA * 128])
    q = rank + secbase[e_sec[order]]          # slot index within group space
    part = q % 128
    chunk = q // 128

    # fill idx + dstl arrays (padded defaults)
    idx_val = np.zeros((NCORE, G, C * 128), np.int16)     # 0 = safe row at base
    dstl = np.full((NCORE, G, 128, C), PAD_DST, np.float32)
    oc = e_core[order]
    og = e_grp[order]
    osec = e_sec[order]
    oslot = e_srcslot[order]
    base = np.where(osec == 0, BASE_A, BASE_B)
    iv = oslot - base
    assert iv.min() >= -32768 and iv.max() <= 32767
    idx_val[oc, og, q] = iv.astype(np.int16)
    dstl[oc, og, part, chunk] = e_dloc[order].astype(np.float32)
    # guarantee the LAST index of each section is >= 0 (gen truncates
    # trailing negatives): pad slots already 0; if a real edge with negative
    # idx lands exactly at the section end, swap it with a pad... instead just
    # check and fix by appending nothing: force last element of each section
    # to be a pad when the section is full AND negative.
    lastA = idx_val[:, :, CA * 128 - 1]
    lastB = idx_val[:, :, C * 128 - 1]
    if (lastA < 0).any() or (lastB < 0).any():
        # swap the offending last element with the first non-negative element
        for c, g in zip(*np.where(lastA < 0)):
            sec = idx_val[c, g, :CA * 128]
            j = int(np.argmax(sec >= 0))
            _swap_edge(idx_val, dstl, c, g, CA * 128 - 1, j)
        for c, g in zip(*np.where(lastB < 0)):
            sec = idx_val[c, g, CA * 128:]
            j = CA * 128 + int(np.argmax(sec >= 0))
            _swap_edge(idx_val, dstl, c, g, C * 128 - 1, j)

    # wrapped int16 layout [128, n/16] (16-partition blocks replicated 8x)
    def wrap(vals):  # vals [..., n] -> [..., 128, n//16]
        n = vals.shape[-1]
        w = vals.reshape(*vals.shape[:-1], n // 16, 16)
        w = np.swapaxes(w, -1, -2)              # [..., 16, n//16]
        return np.tile(w, (1,) * (vals.ndim - 1) + (8, 1))

    gidx = np.ascontiguousarray(wrap(idx_val))            # [NCORE, G, 128, C*8]

    # u1-phase gather: idx-list position i writes tile [p=i%128, c=i//128],
    # and the store DMA maps tile [p, c] -> u1_stage row p*NU+c.
    NU = SLOTS // 128
    ii = np.arange(SLOTS)
    slot_at_i = (ii % 128) * NU + ii // 128
    types_slot = np.zeros((NCORE, SLOTS), np.int64)
    dis_slot = np.zeros((NCORE, SLOTS), np.float32)
    for c in range(NCORE):
        valid = slot2node[c] >= 0
        types_slot[c, valid] = types[slot2node[c, valid]]
        dis_slot[c, valid] = dis[slot2node[c, valid]]
    ut_idx = wrap(types_slot[:, slot_at_i].astype(np.int16))  # [NCORE, 128, SLOTS//16]
    # dis_pc[c][p][j] = dis of slot p*NU+j
    dis_pc = dis_slot[:, (np.arange(128)[:, None] * NU + np.arange(NU)[None, :])]

    disb = np.ascontiguousarray(
        np.broadcast_to(
            dis_slot.reshape(NCORE, G, 1, W), (NCORE, G, 128, W)
        )
    ).astype(np.float32)
    dis_cols = np.ascontiguousarray(
        dis_slot.reshape(NCORE, G, W).transpose(0, 2, 1)
    ).astype(np.float32)                                   # [NCORE, 128, G]

    embW1 = (np.asarray(emb, np.float32) @ np.asarray(W1, np.float32)).astype(np.float32)
    iota_bc = np.tile(np.arange(W, dtype=np.float32)[None, :], (128, 1))
    b1c = np.asarray(b1, np.float32).reshape(128, 1)
    b2r = np.tile(np.asarray(b2, np.float32)[None, :], (128, 1))

    return dict(
        CA=CA, CB=CB, dis=dis, slot2node=slot2node,
        gidx=gidx, dstl=np.ascontiguousarray(dstl.reshape(NCORE, G, 128, C)),
        ut_idx=np.ascontiguousarray(ut_idx),
        dis_pc=np.ascontiguousarray(dis_pc.astype(np.float32)),
        disb=disb, dis_cols=dis_cols,
        embW1=embW1, iota_bc=iota_bc, b1c=b1c, b2r=b2r,
        w2=np.asarray(W2, np.float32),
    )


def _swap_edge(idx_val, dstl, c, g, i, j):
    C = dstl.shape[-1]
    idx_val[c, g, i], idx_val[c, g, j] = idx_val[c, g, j], idx_val[c, g, i]
    pi, ci, pj, cj = i % 128, i // 128, j % 128, j // 128
    t = dstl[c, g, pi, ci]
    dstl[c, g, pi, ci] = dstl[c, g, pj, cj]
    dstl[c, g, pj, cj] = t


# ---------------------------------------------------------------- device
def build_program(CA, CB):
    from concourse import bacc, mybir, tile

    C = CA + CB
    f32, i16 = mybir.dt.float32, mybir.dt.int16
    NU = SLOTS // 128

    nc = bacc.Bacc(None, target_bir_lowering=False, num_devices=NCORE,
                   num_swdge_queues=4)
    embw1_in = nc.dram_tensor("embw1", [NTYPES, D1], f32, kind="ExternalInput")
    w2_in = nc.dram_tensor("w2", [D1, D2], f32, kind="ExternalInput")
    gidx_in = nc.dram_tensor("gidx", [G, 128, C * 8], i16, kind="ExternalInput")
    dstl_in = nc.dram_tensor("dstl", [G, 128, C], f32, kind="ExternalInput")
    utidx_in = nc.dram_tensor("utidx", [128, SLOTS // 16], i16, kind="ExternalInput")
    dispc_in = nc.dram_tensor("dispc", [128, NU], f32, kind="ExternalInput")
    disb_in = nc.dram_tensor("disb", [G, 128, W], f32, kind="ExternalInput")
    discols_in = nc.dram_tensor("discols", [128, G], f32, kind="ExternalInput")
    iota_in = nc.dram_tensor("iota", [128, W], f32, kind="ExternalInput")
    b1c_in = nc.dram_tensor("b1c", [128, 1], f32, kind="ExternalInput")
    b2r_in = nc.dram_tensor("b2r", [128, D2], f32, kind="ExternalInput")
    out_ext = nc.dram_tensor("out", [SLOTS, D2], f32, kind="ExternalOutput")

    u1_stage = nc.dram_tensor("u1_stage", [SLOTS, D1], f32)
    u1_full = nc.dram_tensor("u1_full", [TOTAL_SLOTS, D1], f32, addr_space="Shared")
    u2_stage = nc.dram_tensor("u2_stage", [SLOTS, D2], f32)
    u2_full = nc.dram_tensor("u2_full", [TOTAL_SLOTS, D2], f32, addr_space="Shared")

    RG = [list(range(NCORE))]
    Relu = mybir.ActivationFunctionType.Relu
    Copy = mybir.ActivationFunctionType.Copy
    Ident = mybir.ActivationFunctionType.Identity

    with tile.TileContext(nc) as tc:
        with tc.tile_pool(name="cst", bufs=1) as cst, \
             tc.tile_pool(name="u1p", bufs=1) as u1p, \
             tc.tile_pool(name="gat", bufs=3) as gatp, \
             tc.tile_pool(name="ohp", bufs=2) as ohp, \
             tc.tile_pool(name="sm", bufs=3) as sm, \
             tc.tile_pool(name="hp", bufs=2) as hp, \
             tc.tile_pool(name="op", bufs=3) as op, \
             tc.tile_pool(name="ps1", bufs=2, space="PSUM") as ps1, \
             tc.tile_pool(name="ps2", bufs=2, space="PSUM") as ps2:

            w2_t = cst.tile([D1, D2], f32)
            nc.sync.dma_start(out=w2_t[:], in_=w2_in[:])
            iota_t = cst.tile([128, W], f32)
            nc.sync.dma_start(out=iota_t[:], in_=iota_in[:])
            b1c_t = cst.tile([128, 1], f32)
            nc.sync.dma_start(out=b1c_t[:], in_=b1c_in[:])
            b2r_t = cst.tile([128, D2], f32)
            nc.sync.dma_start(out=b2r_t[:], in_=b2r_in[:])
            discols_t = cst.tile([128, G], f32)
            nc.sync.dma_start(out=discols_t[:], in_=discols_in[:])
            dispc_t = cst.tile([128, NU], f32)
            nc.sync.dma_start(out=dispc_t[:], in_=dispc_in[:])
            utidx_t = cst.tile([128, SLOTS // 16], i16)
            nc.sync.dma_start(out=utidx_t[:], in_=utidx_in[:])

            # ---- u1 = dis * embW1[type]
            u1t = u1p.tile([128, NU * D1], f32)
            nc.gpsimd.dma_gather(
                out_ap=u1t[:].rearrange("p (c d) -> p c d", d=D1),
                in_ap=embw1_in[:],
                idxs_ap=utidx_t[:],
                num_idxs=SLOTS, num_idxs_reg=SLOTS,
                elem_size=D1, single_packet=False, queue_num=0,
            )
            nc.vector.tensor_tensor(
                out=u1t[:].rearrange("p (c d) -> p c d", d=D1),
                in0=u1t[:].rearrange("p (c d) -> p c d", d=D1),
                in1=dispc_t[:][:, :, None].to_broadcast([128, NU, D1]),
                op=mybir.AluOpType.mult,
            )
            nc.sync.dma_start(
                out=u1_stage[:].rearrange("(p c) d -> p c d", p=128),
                in_=u1t[:].rearrange("p (c d) -> p c d", d=D1),
            )
            nc.gpsimd.collective_compute(
                "AllGather", mybir.AluOpType.bypass, replica_groups=RG,
                ins=[u1_stage[:]], outs=[u1_full[:]],
            )

            # ---- layer 1 groups
            for g in range(G):
                gi = sm.tile([128, C * 8], i16, tag="gi")
                nc.sync.dma_start(out=gi[:], in_=gidx_in[g])
                dl = sm.tile([128, C], f32, tag="dl")
                nc.sync.dma_start(out=dl[:], in_=dstl_in[g])
                db = sm.tile([128, W], f32, tag="db")
                nc.sync.dma_start(out=db[:], in_=disb_in[g])

                gat = gatp.tile([128, C * D1], f32, tag="gat")
                nc.gpsimd.dma_gather(
                    out_ap=gat[:, :CA * D1].rearrange("p (c d) -> p c d", d=D1),
                    in_ap=u1_full[BASE_A:, :],
                    idxs_ap=gi[:, :CA * 8],
                    num_idxs=CA * 128, num_idxs_reg=CA * 128,
                    elem_size=D1, single_packet=False, queue_num=(2 * g) % 4,
                )
                nc.gpsimd.dma_gather(
                    out_ap=gat[:, CA * D1:].rearrange("p (c d) -> p c d", d=D1),
                    in_ap=u1_full[BASE_B:, :],
                    idxs_ap=gi[:, CA * 8:],
                    num_idxs=CB * 128, num_idxs_reg=CB * 128,
                    elem_size=D1, single_packet=False, queue_num=(2 * g + 1) % 4,
                )

                oh = ohp.tile([128, C * W], f32, tag="oh")
                nc.vector.tensor_tensor(
                    out=oh[:].rearrange("p (c w) -> p c w", w=W),
                    in0=dl[:][:, :, None].to_broadcast([128, C, W]),
                    in1=iota_t[:][:, None, :].to_broadcast([128, C, W]),
                    op=mybir.AluOpType.is_equal,
                )

                aggT = ps1.tile([D1, W], f32, space="PSUM", tag="aggT")
                for c in range(C):
                    nc.tensor.matmul(
                        out=aggT[:],
                        lhsT=gat[:, c * D1:(c + 1) * D1],
                        rhs=oh[:, c * W:(c + 1) * W],
                        start=(c == 0), stop=(c == C - 1),
                    )

                h1 = hp.tile([D1, W], f32, tag="h1")
                nc.vector.tensor_tensor(
                    out=h1[:], in0=aggT[:], in1=db[:], op=mybir.AluOpType.mult)
                h1b = hp.tile([D1, W], f32, tag="h1b")
                nc.scalar.activation(h1b[:], h1[:], Relu, bias=b1c_t[:, 0:1],
                                     scale=1.0)
                u2ps = ps2.tile([W, D2], f32, space="PSUM", tag="u2ps")
                nc.tensor.matmul(out=u2ps[:], lhsT=h1b[:], rhs=w2_t[:],
                                 start=True, stop=True)
                u2t = op.tile([W, D2], f32, tag="u2t")
                nc.scalar.activation(u2t[:], u2ps[:], Copy,
                                     scale=discols_t[:, g:g + 1])
                nc.sync.dma_start(out=u2_stage[g * W:(g + 1) * W, :], in_=u2t[:])

            nc.gpsimd.collective_compute(
                "AllGather", mybir.AluOpType.bypass, replica_groups=RG,
                ins=[u2_stage[:]], outs=[u2_full[:]],
            )

            # ---- layer 2 groups
            for g in range(G):
                gi = sm.tile([128, C * 8], i16, tag="gi")
                nc.sync.dma_start(out=gi[:], in_=gidx_in[g])
                dl = sm.tile([128, C], f32, tag="dl")
                nc.sync.dma_start(out=dl[:], in_=dstl_in[g])

                gat2 = gatp.tile([128, C * D2], f32, tag="gat2")
                nc.gpsimd.dma_gather(
                    out_ap=gat2[:, :CA * D2].rearrange("p (c d) -> p c d", d=D2),
                    in_ap=u2_full[BASE_A:, :],
                    idxs_ap=gi[:, :CA * 8],
                    num_idxs=CA * 128, num_idxs_reg=CA * 128,
                    elem_size=D2, single_packet=False, queue_num=(2 * g) % 4,
                )
                nc.gpsimd.dma_gather(
                    out_ap=gat2[:, CA * D2:].rearrange("p (c d) -> p c d", d=D2),
                    in_ap=u2_full[BASE_B:, :],
                    idxs_ap=gi[:, CA * 8:],
                    num_idxs=CB * 128, num_idxs_reg=CB * 128,
                    elem_size=D2, single_packet=False, queue_num=(2 * g + 1) % 4,
                )

                oh = ohp.tile([128, C * W], f32, tag="oh")
                nc.vector.tensor_tensor(
                    out=oh[:].rearrange("p (c w) -> p c w", w=W),
                    in0=dl[:][:, :, None].to_broadcast([128, C, W]),
                    in1=iota_t[:][:, None, :].to_broadcast([128, C, W]),
                    op=mybir.AluOpType.is_equal,
                )

                agg2 = ps2.tile([W, D2], f32, space="PSUM", tag="agg2")
                for c in range(C):
                    nc.tensor.matmul(
                        out=agg2[:],
                        lhsT=oh[:, c * W:(c + 1) * W],
                        rhs=gat2[:, c * D2:(c + 1) * D2],
                        start=(c == 0), stop=(c == C - 1),
                    )

                o1 = op.tile([W, D2], f32, tag="o1")
                nc.scalar.activation(o1[:], agg2[:], Copy,
                                     scale=discols_t[:, g:g + 1])
                o2 = op.tile([W, D2], f32, tag="o2")
                nc.vector.tensor_tensor(
                    out=o2[:], in0=o1[:], in1=b2r_t[:], op=mybir.AluOpType.add)
                nc.sync.dma_start(out=out_ext[g * W:(g + 1) * W, :], in_=o2[:])

    nc.compile()
    return nc


def kernel(x_node_types, edge_index, emb, W1, b1, W2, b2):
    from concourse.bass_utils import run_bass_kernel_spmd

    pre = preprocess(x_node_types, edge_index, emb, W1, b1, W2, b2)
    nc = build_program(pre["CA"], pre["CB"])

    in_maps = []
    for c in range(NCORE):
        in_maps.append({
            "embw1": pre["embW1"], "w2": pre["w2"],
            "gidx": pre["gidx"][c], "dstl": pre["dstl"][c],
            "utidx": pre["ut_idx"][c], "dispc": pre["dis_pc"][c],
            "disb": pre["disb"][c], "discols": pre["dis_cols"][c],
            "iota": pre["iota_bc"], "b1c": pre["b1c"], "b2r": pre["b2r"],
        })

    trace = bool(int(os.environ.get("BASS_KERNEL_TRACE", "0")))
    res = run_bass_kernel_spmd(nc, in_maps, list(range(NCORE)), trace=trace)
    if trace and res.exec_time_ns is not None:
        print(f"HW exec time: {res.exec_time_ns} ns")

    out = np.zeros((N_NODES, D2), np.float32)
    s2n = pre["slot2node"]
    for c in range(NCORE):
        valid = s2n[c] >= 0
        out[s2n[c, valid]] = res.results[c]["out"][valid]
    return out
